# revision 55
# baseline (speedup 1.0000x reference)
"""Trainium2 Bass kernel for batched self-attention with input projections.

Problem: B=8, N=2048, D=131
    Q = q @ Wq.T + bq;  K = k @ Wk.T + bk;  V = v @ Wv.T + bv
    out = softmax(Q K^T / sqrt(131)) V

One batch element per NeuronCore (8 cores, no communication).

Host prep (layout/algebra only):
  - Tokens augmented with a ones-row: X = [x^T; 1] in [132, 2048] so biases
    fold into the projection matmuls.
  - Scores: Q K^T = Xq (Wq'^T Wk'/sqrt(D)) Xk^T = Xq G Xk^T, G [132,132].
    SVD-truncate G to rank 128 (exact rank 131; error ~2e-5) so the big S
    matmul is a single K=128 contraction:  S = (Xq Aq)(Xk Ak)^T.
  - Value path: W2 [132,132] maps X -> [V | 1] (bias row + denominator
    ones-column).  SVD-truncate W2 = L R^T to rank 128 so the O-matmul
    contracts into a 128-wide latent:  O' = (P Xv L) R^T, with O'[:,131]
    the softmax denominator.
  - Projections/S in bf16 (fp32 PSUM accumulation); the O path (exp
    weights E and the value latent VL) in fp8e4m3 so the O accumulation
    runs in DoubleRow perf mode: each matmul contracts TWO 128-token
    k-tiles at once (lhsT [128,2,128], rhs [128,2,1024]), halving the PE
    time of the biggest accumulation.  Measured rel err ~1.05e-2 (numpy
    bit-accurate sim of the dtype pipeline).  |S| < 3 so softmax without
    max-subtraction is safe.

Per core:
  QT[e',n] = Aq^T Xq, KT[e',n] = Ak^T Xk      (hi[128]+lo[4] d-chunks;
      the lo rows ride the xlo4 quarter layout: partition bases {0,32,64}
      via PE tile_position, so their DMA is not 4-partition-bw-bound)
  VL[n,l]  = Xv^T L -> fp8 pair tiles [128,2,128] (j, j+1)
  for i-half h (1024 cols), j-block (16):
      ST = KT_j^T QT_h  (2x512-col matmuls: PSUM-bank limit) -> one
      [128,1024] exp on ACT -> fp8 E pair tiles [128,2,1024]; every 2 j's
      one DoubleRow matmul pair accumulates Ohat^T[l, h] in PSUM.
  O'[i,132] = Ohat_i R^T;  out = O'[:,0:131] / O'[:,131]

Schedule notes (all measured on HW):
  - ACT does exp only during the stream (exp [128,1024] fp32->fp8 =
    1114ns; smaller granules pay ~470 cycles fixed per ACTIVATE).  The
    activation table preloads via a dummy exp during the DMA window.
  - A 10-matmul junk burst opens the HAM clock gate early; without it
    the PE runs at ~0.85GHz for the first ~30us.
  - DMA issues cost ~650ns on the issuing engine queue and each engine
    owns one HW queue; issues are spread sync/scalar/gpsimd with the
    first-exp-critical bytes (Q half 0, K chunk 0, lo quarter 0) first.
  - O-pairs trail their exps by two pairs in the in-order PE queue so
    their exp-completion waits never stall the next S matmuls; VL
    projections run at single-j-block granule so each fits an inter-pair
    PE slack slot.
  - h0 finalization rides the late-h1 exp-wait slack on PE (putting it
    mid-stream steals exp pace); h1 finalization pipelines ohat copies
    in 256-col pieces alternating ACT/DVE, normalizes on both engines,
    and rotates po PSUM through the freed score banks.
  - A post-finalize pass drops Ldweights that reload identical weights.
"""

import numpy as np
import ml_dtypes

P = 128          # partitions / PE width
N = 2048         # tokens per core
D = 131          # embed dim
DP = 132         # embed dim + ones row
DLO = DP - P     # tail contraction rows (4)
R = 128          # truncated rank (QK interaction and V latent)
EV = 132         # final output cols (131 + denominator)
NB = N // P      # 16 token blocks
NPAIR = NB // 2  # 8 j-block pairs (DoubleRow granule)
HW = 1024        # i-half width
NH = N // HW     # 2 halves
NCORES = 8

QOFF, KOFF, VOFF = 0, N, 2 * N          # column offsets in packed xall
QBASE = (0, 32, 64, 0)   # lo-quarter partition bases (AP supports 0/32/64)
QCOL = (0, 0, 0, 1536)   # lo-quarter column offsets in xlo
AQOFF, AKOFF, LOFF = 0, R, 2 * R        # column offsets in packed weights

_BF16 = ml_dtypes.bfloat16


def build_nc():
    """Build the single-core Bass graph (same NEFF runs SPMD on all 8 cores)."""
    from contextlib import ExitStack

    import concourse.bacc as bacc
    import concourse.mybir as mybir
    import concourse.tile as tile
    from concourse.bass import ts

    bf = mybir.dt.bfloat16
    f8 = mybir.dt.float8e4
    f32 = mybir.dt.float32
    EXP = mybir.ActivationFunctionType.Exp
    COPY = mybir.ActivationFunctionType.Copy
    DR = mybir.MatmulPerfMode.DoubleRow

    nc = bacc.Bacc()
    xall = nc.declare_dram_parameter("xall", [P, 3 * N], bf, isOutput=False)
    # lo rows (x dims 128-130 + ones) packed on partition bases {0,32,64,96}
    # (one 512-col quarter per base): 4x the partition-parallel DMA bw vs a
    # [4, 6144] layout, whose 4-partition writes run at ~1.2GB/s/partition.
    # Layout: xlo4[32q+r, T*512+c] = X_T[128+r, 512q+c] for tensors T=q,k,v.
    xlo4 = nc.declare_dram_parameter("xlo4", [P, 3584], bf, isOutput=False)
    wpack = nc.declare_dram_parameter("wpack", [P, 3 * R], bf, isOutput=False)
    # wpack lo rows replicated at each partition base (tiny)
    wplo4 = nc.declare_dram_parameter("wplo4", [P, 3 * R], bf, isOutput=False)
    rmat = nc.declare_dram_parameter("rmat", [R, EV], bf, isOutput=False)
    out = nc.declare_dram_parameter("out", [N, D], f32, isOutput=True)

    with tile.TileContext(nc) as tc, ExitStack() as ctx:
        const = ctx.enter_context(tc.tile_pool(name="const", bufs=1))
        xin = ctx.enter_context(tc.tile_pool(name="xin", bufs=1))
        proj = ctx.enter_context(tc.tile_pool(name="proj", bufs=1))
        vpool = ctx.enter_context(tc.tile_pool(name="vpool", bufs=1))
        epool = ctx.enter_context(tc.tile_pool(name="epool", bufs=6))
        ebp = ctx.enter_context(tc.tile_pool(name="ebp", bufs=4))
        ohs = ctx.enter_context(tc.tile_pool(name="ohs", bufs=1))
        outp = ctx.enter_context(tc.tile_pool(name="outp", bufs=4))
        warm = ctx.enter_context(tc.tile_pool(name="warm", bufs=1))
        # PSUM budget (8 banks): proj/final 2 x [128,512] = 2, scores
        # 2 x [128,1024] = 4, Ohat accumulator 1 x [128,1024] = 2.
        psp = ctx.enter_context(tc.tile_pool(name="psp", bufs=2, space="PSUM"))
        psst = ctx.enter_context(tc.tile_pool(name="psst", bufs=2, space="PSUM"))
        psoh = ctx.enter_context(tc.tile_pool(name="psoh", bufs=1, space="PSUM"))

        # ---- DMA loads.  Column-chunked [128, 1024] transfers (one 2D
        # descriptor each, 2KB per partition line).  Each dma_start costs
        # ~650ns of issue time on its engine's queue, so the issues are
        # spread across engines: sync takes the critical-path Q/K/V order,
        # gpsimd (otherwise idle) takes the weights + tail rows, vector
        # takes one K chunk after its memset.
        wp_hi = const.tile([P, 3 * R], bf)
        wp_lo4 = const.tile([P, 3 * R], bf)
        xall_hi = xin.tile([P, 3 * N], bf)
        xlo = xin.tile([P, 3584], bf)
        rmat_s = const.tile([R, EV], bf)
        # Transfer priority: per-partition DMA write bw is ~1.2-2GB/s, so
        # what matters is bytes-per-partition ahead of each need.  The
        # 4-partition tail rows are extra slow and serialize with each
        # other, so they are split per-half and fronted.  sync carries the
        # 128-partition chunks in first-use order; scalar (idle pre-stream)
        # carries the weights + tail rows.
        nc.sync.dma_start(
            out=xall_hi[:, QOFF:QOFF + 512], in_=xall[:, QOFF:QOFF + 512]
        )
        nc.sync.dma_start(
            out=xall_hi[:, KOFF:KOFF + 512], in_=xall[:, KOFF:KOFF + 512]
        )
        # K-lo quarter 0 lives at base 64 (own columns) so it transfers in
        # parallel with the base-0 Q-lo quarter on the scalar queue
        nc.sync.dma_start(
            out=xlo[64:68, 3072:3584], in_=xlo4[64:68, 3072:3584]
        )
        nc.sync.dma_start(
            out=xall_hi[:, QOFF + 512:QOFF + HW],
            in_=xall[:, QOFF + 512:QOFF + HW],
        )
        nc.sync.dma_start(
            out=xall_hi[:, KOFF + 512:KOFF + HW],
            in_=xall[:, KOFF + 512:KOFF + HW],
        )
        for lo, hi in (
            (VOFF, VOFF + HW),           # V blocks 0-7
            (VOFF + HW, VOFF + N),       # V blocks 8-15
            (KOFF + HW, KOFF + N),       # K chunks 2-3
            (QOFF + HW, QOFF + N),       # Q half 1
        ):
            nc.sync.dma_start(
                out=xall_hi[:, lo:hi], in_=xall[:, lo:hi]
            )
        wsrc = warm.tile([P, 512], bf)
        nc.vector.memset(wsrc, 0)
        wdum = warm.tile([P, 4], bf)
        # scalar carries only the first-exp-critical small transfers so the
        # exp stream is not stuck behind DMA issues on the ACT queue
        nc.scalar.dma_start(out=wp_hi, in_=wpack[:, :])
        nc.scalar.dma_start(out=wp_lo4, in_=wplo4[:, :])
        nc.scalar.dma_start(out=xlo[0:4, 0:512], in_=xlo4[0:4, 0:512])
        nc.scalar.activation(wdum, wsrc[:, 0:4], EXP)
        # remaining lo quarters (disjoint partition bases transfer in
        # parallel) + rmat ride the otherwise-idle gpsimd queue
        nc.gpsimd.dma_start(
            out=xlo[32:36, 0:1536], in_=xlo4[32:36, 0:1536]
        )
        nc.gpsimd.dma_start(out=rmat_s, in_=rmat[:, :])
        nc.gpsimd.dma_start(out=xlo[0:4, 512:1024], in_=xlo4[0:4, 512:1024])
        nc.gpsimd.dma_start(
            out=xlo[0:4, 1536:3072], in_=xlo4[0:4, 1536:3072]
        )
        nc.gpsimd.dma_start(out=xlo[0:4, 1024:1536], in_=xlo4[0:4, 1024:1536])
        nc.gpsimd.dma_start(
            out=xlo[64:68, 0:1536], in_=xlo4[64:68, 0:1536]
        )

        # ---- junk-matmul burst: a long dense burst is REQUIRED to open
        # the HAM clock gate early (~13us); without it the whole first half
        # of the kernel runs at ~0.85GHz.  Later stages gate on the input
        # DMAs so the burst tracks transfer progress.
        for w in range(6):
            pw = psst.tile([P, HW], f32, tag="pst", name="pw")
            nc.tensor.matmul(pw[:, 0:512], wsrc[:, 0:P], wsrc, start=True, stop=True)
        for w in range(4):
            pw = psst.tile([P, HW], f32, tag="pst", name="pw2")
            nc.tensor.matmul(pw[:, 0:512], wp_hi[:, 0:P], wsrc, start=True, stop=True)

        def junk(n=1):
            for w in range(n):
                pw = psst.tile([P, HW], f32, tag="pst", name="pwj")
                nc.tensor.matmul(
                    pw[:, 0:512], wsrc[:, 0:P], wsrc, start=True, stop=True
                )

        # ---- projection tiles.  QT merged per half so each S_j is ONE
        # [128,1024] matmul; KT chunked [128,512]; VL as fp8 pair tiles.
        qth = [proj.tile([P, HW], bf, tag=f"qh{h}", name=f"qh{h}") for h in range(NH)]
        kts = [proj.tile([P, 512], bf, tag=f"kt{c}", name=f"kt{c}") for c in range(4)]
        vps = [vpool.tile([P, 2, P], f8, tag=f"vp{g}", name=f"vp{g}")
               for g in range(NPAIR)]

        def qk_chunk(dst, woff, xoff, c, on_psst=False, jmid=0, act_copy=False):
            if on_psst:
                ppt = psst.tile([P, HW], f32, tag="pst", name="ppk")
                pp = ppt[:, 0:512]
            else:
                pp = psp.tile([P, 512], f32, tag="pp", name="pp")
            nc.tensor.matmul(
                pp,
                wp_hi[:, woff:woff + R],
                xall_hi[:, xoff + c * 512: xoff + (c + 1) * 512],
                start=True,
                stop=False,
            )
            junk(jmid)
            t = xoff // N
            if t == 1 and c == 0:
                b, col = 64, 3072
            else:
                b, col = QBASE[c], QCOL[c] + t * 512
            nc.tensor.matmul(
                pp,
                wp_lo4[b:b + DLO, woff:woff + R],
                xlo[b:b + DLO, col:col + 512],
                start=False,
                stop=True,
            )
            if act_copy:
                nc.scalar.activation(dst, pp, COPY)
            else:
                nc.vector.tensor_copy(dst, pp)

        vgrp = [None]

        def vl_quarter(j):
            """Project VL for one j-block into its fp8 pair-tile slot.
            Single-block granule (~0.65us) fits an inter-pair PE slack;
            four blocks share one PSUM tile so the psp pool rotation (which
            waits on DVE copy completions) turns over 4x less often."""
            if j % 4 == 0:
                vgrp[0] = psp.tile([P, 512], f32, tag="pp", name="pv")
            pv = vgrp[0][:, ts(j % 4, P)]
            g4 = j // 4
            b = QBASE[g4]
            tq = j % 4
            nc.tensor.matmul(
                pv,
                xall_hi[:, VOFF + j * P: VOFF + (j + 1) * P],
                wp_hi[:, LOFF:LOFF + R],
                start=True,
                stop=False,
            )
            nc.tensor.matmul(
                pv,
                xlo[b:b + DLO,
                    QCOL[g4] + 1024 + tq * P:QCOL[g4] + 1024 + (tq + 1) * P],
                wp_lo4[b:b + DLO, LOFF:LOFF + R],
                start=False,
                stop=True,
            )
            nc.vector.tensor_copy(vps[j // 2][:, j % 2, :], pv)

        def s_exp(h, j, edst):
            """S^T_j for half h; 512-col granule for both the matmuls
            (PSUM-bank limit) and the exps (same ACT cost, half the
            S->exp latency)."""
            pst = psst.tile([P, HW], f32, tag="pst", name="pst")
            for c in range(2):
                nc.tensor.matmul(
                    pst[:, ts(c, 512)],
                    kts[j // 4][:, ts(j % 4, P)],
                    qth[h][:, ts(c, 512)],
                    start=True,
                    stop=True,
                )
            nc.scalar.activation(edst, pst, EXP)

        def o_pair(poh, g, ep):
            """DoubleRow fp8 matmuls: contract j-blocks 2g and 2g+1 at once."""
            for c in range(2):
                nc.tensor.matmul(
                    poh[:, ts(c, 512)],
                    vps[g],
                    ep[:, :, ts(c, 512)],
                    start=(g == 0),
                    stop=(g == NPAIR - 1),
                    perf_mode=DR,
                )

        def finalize_group(h, g, ohat, act_mul=False):
            """Two i-blocks -> O' = Ohat R^T, normalize, DMA out.  act_mul
            puts one of the two normalizes on ACT and the output DMA issue
            on the scalar queue (post-exp-stream only)."""
            stage = outp.tile([P, 2, D], f32, tag="stage", name="stage")
            for t in range(2):
                i = 2 * g + t
                if act_mul and t == 0:
                    # post-stream the psst banks are free: 4-deep rotation
                    pot = psst.tile([P, HW], f32, tag="pst", name="pof")
                    po = pot[:, 0:EV]
                else:
                    po = psp.tile([P, EV], f32, tag="pp", name="po")
                nc.tensor.matmul(
                    po, ohat[:, ts(i % 8, P)], rmat_s, start=True, stop=True
                )
                rec = outp.tile([P, 1], f32, tag="rec", name="rec")
                nc.vector.reciprocal(rec, po[:, D:D + 1])
                if act_mul and t == 1:
                    nc.scalar.activation(
                        stage[:, t, :], po[:, 0:D], COPY, scale=rec
                    )
                else:
                    nc.vector.tensor_scalar_mul(stage[:, t, :], po[:, 0:D], rec)
            nc.sync.dma_start(
                    out=out[g * 256:(g + 1) * 256, :].rearrange(
                        "(t p) e -> p t e", p=P
                    ),
                    in_=stage,
                )

        # ---- h=0 stream: project what each j needs just in time, start
        # the exp stream as early as possible, trail it with the DoubleRow
        # O accumulation; VL groups fill PE slack between S matmuls.
        qk_chunk(qth[0][:, 0:512], AQOFF, QOFF, 0, jmid=1)
        qk_chunk(kts[0], AKOFF, KOFF, 0, on_psst=True, jmid=1, act_copy=True)
        qk_chunk(qth[0][:, 512:HW], AQOFF, QOFF, 1, act_copy=True)

        poh0 = psoh.tile([P, HW], f32, tag="poh", name="poh0")
        eps0 = []

        def h0_step(j):
            if j % 2 == 0:
                ep = epool.tile([P, 2, HW], f8, tag="e", name=f"e0_{j // 2}")
                eps0.append(ep)
            s_exp(0, j, eps0[j // 2][:, j % 2, :])

        # O-pairs trail their exps by TWO pairs in the PE queue: an
        # o_pair placed right after S(2g+1) waits on exp(2g+1) and stalls
        # the in-order PE queue (~100ns/tile of exp-stream pace).
        h0_step(0)
        h0_step(1)
        qk_chunk(kts[1], AKOFF, KOFF, 1)
        h0_step(2)
        vl_quarter(0)
        h0_step(3)
        vl_quarter(1)
        h0_step(4)
        vl_quarter(2)
        h0_step(5)
        o_pair(poh0, 0, eps0[0])
        vl_quarter(3)
        h0_step(6)
        qk_chunk(kts[2], AKOFF, KOFF, 2)
        h0_step(7)
        o_pair(poh0, 1, eps0[1])
        vl_quarter(4)
        h0_step(8)
        vl_quarter(5)
        h0_step(9)
        o_pair(poh0, 2, eps0[2])
        vl_quarter(6)
        h0_step(10)
        qk_chunk(kts[3], AKOFF, KOFF, 3)
        vl_quarter(7)
        h0_step(11)
        o_pair(poh0, 3, eps0[3])
        vl_quarter(8)
        h0_step(12)
        vl_quarter(9)
        h0_step(13)
        o_pair(poh0, 4, eps0[4])
        vl_quarter(10)
        h0_step(14)
        qk_chunk(qth[1][:, 0:512], AQOFF, QOFF, 2)
        vl_quarter(11)
        h0_step(15)
        o_pair(poh0, 5, eps0[5])
        vl_quarter(12)
        qk_chunk(qth[1][:, 512:HW], AQOFF, QOFF, 3)
        vl_quarter(13)

        # ---- h=1 stream with h=0 finalization interleaved.
        poh1 = psoh.tile([P, HW], f32, tag="poh", name="poh1")
        eps1 = []

        def h1_step(j):
            if j % 2 == 0:
                ep = epool.tile([P, 2, HW], f8, tag="e", name=f"e1_{j // 2}")
                eps1.append(ep)
            s_exp(1, j, eps1[j // 2][:, j % 2, :])

        h1_step(0)
        vl_quarter(14)
        h1_step(1)
        o_pair(poh0, 6, eps0[6])
        vl_quarter(15)
        h1_step(2)
        h1_step(3)
        o_pair(poh0, 7, eps0[7])
        ohat0 = ohs.tile([P, HW], bf, tag="oh0", name="oh0")
        nc.vector.tensor_copy(ohat0, poh0)
        for j in range(4, NB):
            h1_step(j)
            if j % 2 == 1:
                g = j // 2 - 2
                o_pair(poh1, g, eps1[g])
                if 1 <= g <= 4:
                    # h0 finals ride the late-h1 exp-wait slack
                    finalize_group(0, g - 1, ohat0)
        o_pair(poh1, 6, eps1[6])
        o_pair(poh1, 7, eps1[7])
        ohat1 = ohs.tile([P, HW], bf, tag="oh1", name="oh1")
        # ACT is free once the exp stream ends; copy in 256-col pieces
        # alternating ACT/DVE so finalize group g can start on piece g
        for p4 in range(4):
            if p4 % 2 == 0:
                nc.scalar.activation(
                    ohat1[:, ts(p4, 256)], poh1[:, ts(p4, 256)], COPY
                )
            else:
                nc.vector.tensor_copy(ohat1[:, ts(p4, 256)], poh1[:, ts(p4, 256)])
            finalize_group(1, 4 + p4, ohat1, act_mul=True)

    return nc


def dedup_ldweights(nc):
    """Drop Ldweights instructions that reload the exact weights already in
    the PE array (same AP, nothing clobbering in between).  The PE keeps the
    stationary operand across matmuls, so a back-to-back identical reload is
    pure dispatch overhead (~107ns each).  Only sync-free Ldweights are
    dropped so semaphore ordering is untouched."""
    dropped = 0
    for f in nc.m.functions:
        for blk in f.blocks:
            insts = list(blk.instructions)
            kept = []
            last_key = None
            for ins in insts:
                tname = type(ins).__name__
                if "PE" in str(getattr(ins, "engine", "")):
                    if tname == "InstLdweights":
                        ap = ins.ins[0]
                        key = (
                            ap.memref,
                            ap.offset,
                            str(ap.ap),
                            str(ap.dtype),
                            str(getattr(ins, "is_transpose", None)),
                        )
                        si = ins.sync_info
                        no_sync = si is None or (
                            len(si.on_wait) == 0 and len(si.on_update) == 0
                        )
                        if key == last_key and no_sync:
                            dropped += 1
                            continue
                        last_key = key
                    elif tname not in (
                        "InstMatmult",
                        "InstEventSemaphore",
                        "InstNoOp",
                        "InstDrain",
                    ):
                        last_key = None
                kept.append(ins)
            if len(kept) != len(insts):
                blk.instructions = kept
    return dropped


def prep_host(query, key, value, Wq, bq, Wk, bk, Wv, bv):
    """Host-side layout/algebra prep. Returns per-core input maps."""
    s = np.sqrt(np.float64(D))
    Wqp = np.concatenate([Wq, bq[:, None]], axis=1)  # [131, 132]
    Wkp = np.concatenate([Wk, bk[:, None]], axis=1)
    G = (Wqp.astype(np.float64).T @ Wkp.astype(np.float64)) / s  # [132, 132]
    U, S, Vt = np.linalg.svd(G)
    Aq = (U[:, :R] * np.sqrt(S[:R])).astype(np.float32)  # [132, 128]
    Ak = (Vt[:R, :].T * np.sqrt(S[:R])).astype(np.float32)

    W2 = np.zeros((DP, EV), np.float64)  # maps X -> [V | 1]
    W2[:D, :D] = Wv.T
    W2[D, :D] = bv
    W2[D, D] = 1.0
    U2, S2, V2t = np.linalg.svd(W2)
    L = (U2[:, :R] * np.sqrt(S2[:R])).astype(np.float32)  # [132, 128]
    Rm = (V2t[:R, :].T * np.sqrt(S2[:R])).astype(np.float32)  # [132, 128]

    wpack = np.concatenate([Aq, Ak, L], axis=1)  # [132, 384]
    wpack16 = np.ascontiguousarray(wpack[0:P].astype(_BF16))
    # lo rows replicated at partition bases {0,32,64,96}
    wplo4 = np.zeros((P, 3 * R), np.float32)
    for b in (0, 32, 64):
        wplo4[b:b + DLO] = wpack[P:DP]
    wplo4_16 = np.ascontiguousarray(wplo4.astype(_BF16))
    rmat16 = np.ascontiguousarray(Rm.T.astype(_BF16))  # [128, 132]

    ones_row = np.ones((1, N), np.float32)
    in_maps = []
    for c in range(NCORES):
        xs = [np.concatenate([x.T, ones_row], axis=0)
              for x in (query[c], key[c], value[c])]
        xallc = np.concatenate(xs, axis=1)  # [132, 6144]
        # lo rows: quarter q (cols 512q:512q+512 of each tensor) at
        # partition base 32q; columns [Q 512 | K 512 | V 512]
        xlo4 = np.zeros((P, 3584), np.float32)
        for q in range(4):
            for t in range(3):
                xlo4[QBASE[q]:QBASE[q] + DLO,
                     QCOL[q] + t * 512:QCOL[q] + (t + 1) * 512] = \
                    xallc[P:DP, t * N + 512 * q: t * N + 512 * (q + 1)]
        # K-lo quarter 0 duplicated at base 64 (parallel first transfer)
        xlo4[64:64 + DLO, 3072:3584] = xallc[P:DP, N:N + 512]
        in_maps.append({
            "xall": np.ascontiguousarray(xallc[0:P].astype(_BF16)),
            "xlo4": np.ascontiguousarray(xlo4.astype(_BF16)),
            "wpack": wpack16,
            "wplo4": wplo4_16,
            "rmat": rmat16,
        })
    return in_maps


_NC_CACHE = {}


def _get_nc():
    if "nc" not in _NC_CACHE:
        nc = build_nc()
        if not nc.is_finalized():
            nc.finalize()  # Bacc.finalize runs the wait-split/EVSEM passes
        dedup_ldweights(nc)
        _NC_CACHE["nc"] = nc
    return _NC_CACHE["nc"]


def run_on_cores(in_maps, trace=False, **kw):
    from concourse.bass_utils import run_bass_kernel_spmd

    nc = _get_nc()
    return run_bass_kernel_spmd(nc, in_maps, core_ids=list(range(NCORES)),
                                trace=trace, **kw)


def kernel(query, key, value, Wq, bq, Wk, bk, Wv, bv):
    in_maps = prep_host(query, key, value, Wq, bq, Wk, bk, Wv, bv)
    res = run_on_cores(in_maps)
    return np.stack([np.asarray(res.results[c]["out"]) for c in range(NCORES)])
